# revision 2
# baseline (speedup 1.0000x reference)
"""ForgetMult (h_t = f_t*h_{t-1} + (1-f_t)*z_t) on 8 TRN2 NeuronCores.

Full inputs f, z: [T=1024, B=32, H=1024] f32. Output h: [T, B, H].

Sharding: batch dim across the 8 cores (4 batches/core), no communication.
Per core the problem is [T=1024, N=4096] with an independent linear
recurrence along T for each of the N columns.

Per-core dataflow:
  - DMA f, z in natural [t-partition, n-free] panels (contiguous 2 KiB rows)
  - DVE: bneg = (f - 1) * z   (= -(1-f)*z, one scalar_tensor_tensor op)
  - PE: transpose 128x128 blocks of f and bneg into PSUM -> [n-part, t-free]
  - ACT: copy f_tr PSUM->SBUF (scan operands can't both live in PSUM)
  - DVE: tensor_tensor_scan along t: state = f*state - bneg
         (op0=mult, op1=subtract) == f*state + (1-f)*z
  - PE: transpose h back to [t-part, n-free]; ACT: copy PSUM->SBUF
  - DMA h panels out
"""

import os
import sys
from contextlib import ExitStack

import numpy as np

T, B, H = 1024, 32, 1024
NCORES = 8
BPC = B // NCORES  # 4 batches per core
N = BPC * H  # 4096 recurrence columns per core
P = 128

# Tunables for the full-size build
W_FULL = 512  # panel width (columns per n-group)
SCAN_LEN_FULL = 512  # scan chunk along T (chained); 1024 = single scan
PSUM_TR_BUFS = 2  # bufs for the f_tr / b_tr PSUM pools


def build_forget_mult(tc, h_d, f_d, z_d, i_d, ctx, t_sz, n_sz, w_sz, scan_len):
    """Emit the per-core Tile program. All APs are DRAM [t_sz, n_sz]."""
    import concourse.bass as bass
    from concourse import mybir

    nc = tc.nc
    fp32 = mybir.dt.float32
    su = mybir.AluOpType.subtract
    mu = mybir.AluOpType.mult

    tb = t_sz // P  # t-blocks
    ng = n_sz // w_sz  # n-groups
    nb = w_sz // P  # n-blocks per group
    n_halves = t_sz // scan_len  # scan chunks along T
    thb = scan_len // P  # t-blocks per scan chunk
    assert t_sz % P == 0 and n_sz % w_sz == 0 and w_sz % P == 0
    assert t_sz % scan_len == 0 and scan_len % P == 0

    const_pool = ctx.enter_context(tc.tile_pool(name="const", bufs=1))
    ident = const_pool.tile([P, P], fp32)
    nc.sync.dma_start(ident[:], i_d[:])

    f_pool = ctx.enter_context(tc.tile_pool(name="fpanel", bufs=2 * tb))
    b_pool = ctx.enter_context(tc.tile_pool(name="bpanel", bufs=2 * tb))
    h_pool = ctx.enter_context(tc.tile_pool(name="hpanel", bufs=2 * tb))
    z_pool = ctx.enter_context(tc.tile_pool(name="zstream", bufs=3))
    ftr_s_pool = ctx.enter_context(tc.tile_pool(name="ftrs", bufs=2))
    htr_pool = ctx.enter_context(tc.tile_pool(name="htr", bufs=3))
    ftr_p_pool = ctx.enter_context(
        tc.tile_pool(name="ftrp", bufs=PSUM_TR_BUFS, space="PSUM")
    )
    btr_p_pool = ctx.enter_context(
        tc.tile_pool(name="btrp", bufs=PSUM_TR_BUFS, space="PSUM")
    )
    hbk_p_pool = ctx.enter_context(tc.tile_pool(name="hbkp", bufs=4, space="PSUM"))

    for g in range(ng):
        fts, bts, hts = [], [], []
        for j in range(tb):
            ft = f_pool.tile([P, w_sz], fp32, tag="fpanel")
            nc.sync.dma_start(
                ft[:], f_d[P * j : P * (j + 1), w_sz * g : w_sz * (g + 1)]
            )
            zt = z_pool.tile([P, w_sz], fp32, tag="zstream")
            nc.sync.dma_start(
                zt[:], z_d[P * j : P * (j + 1), w_sz * g : w_sz * (g + 1)]
            )
            bt = b_pool.tile([P, w_sz], fp32, tag="bpanel")
            # bneg = (f - 1) * z
            nc.vector.scalar_tensor_tensor(bt[:], ft[:], 1.0, zt[:], op0=su, op1=mu)
            ht = h_pool.tile([P, w_sz], fp32, tag="hpanel")
            fts.append(ft)
            bts.append(bt)
            hts.append(ht)

        for i in range(nb):
            prev_tail = None
            for half in range(n_halves):
                ftr_p = ftr_p_pool.tile([P, scan_len], fp32, tag="ftrp")
                btr_p = btr_p_pool.tile([P, scan_len], fp32, tag="btrp")
                for jj in range(thb):
                    j = half * thb + jj
                    nc.tensor.transpose(
                        ftr_p[:, P * jj : P * (jj + 1)],
                        fts[j][:, P * i : P * (i + 1)],
                        ident[:],
                    )
                    nc.tensor.transpose(
                        btr_p[:, P * jj : P * (jj + 1)],
                        bts[j][:, P * i : P * (i + 1)],
                        ident[:],
                    )
                ftr_s = ftr_s_pool.tile([P, scan_len], fp32, tag="ftrs")
                nc.scalar.copy(ftr_s[:], ftr_p[:])
                htr = htr_pool.tile([P, scan_len], fp32, tag="htr")
                init = 0.0 if half == 0 else prev_tail
                # state = (f * state) - bneg == f*state + (1-f)*z
                nc.vector.tensor_tensor_scan(
                    htr[:], ftr_s[:], btr_p[:], init, op0=mu, op1=su
                )
                prev_tail = htr[:, scan_len - 1 : scan_len]
                for jj in range(thb):
                    j = half * thb + jj
                    hbk = hbk_p_pool.tile([P, P], fp32, tag="hbkp")
                    nc.tensor.transpose(
                        hbk[:], htr[:, P * jj : P * (jj + 1)], ident[:]
                    )
                    nc.scalar.copy(hts[j][:, P * i : P * (i + 1)], hbk[:])

        for j in range(tb):
            nc.sync.dma_start(
                h_d[P * j : P * (j + 1), w_sz * g : w_sz * (g + 1)], hts[j][:]
            )


def build_program(t_sz=T, n_sz=N, w_sz=W_FULL, scan_len=SCAN_LEN_FULL):
    import concourse.tile as tile
    from concourse import bacc, mybir

    nc = bacc.Bacc(
        "TRN2",
        target_bir_lowering=False,
        debug=False,
        enable_asserts=False,
        num_devices=NCORES,
    )
    fp32 = mybir.dt.float32
    f_d = nc.dram_tensor("f", [t_sz, n_sz], fp32, kind="ExternalInput").ap()
    z_d = nc.dram_tensor("z", [t_sz, n_sz], fp32, kind="ExternalInput").ap()
    i_d = nc.dram_tensor("ident", [P, P], fp32, kind="ExternalInput").ap()
    h_d = nc.dram_tensor("h", [t_sz, n_sz], fp32, kind="ExternalOutput").ap()
    with tile.TileContext(nc) as tc:
        with ExitStack() as ctx:
            build_forget_mult(tc, h_d, f_d, z_d, i_d, ctx, t_sz, n_sz, w_sz, scan_len)
    nc.compile()
    return nc


_compiled = None


def _get_program():
    global _compiled
    if _compiled is None:
        _compiled = build_program()
    return _compiled


def kernel(f, z, _trace=False):
    from concourse.bass_utils import run_bass_kernel_spmd

    f = np.asarray(f, dtype=np.float32)
    z = np.asarray(z, dtype=np.float32)
    assert f.shape == (T, B, H) and z.shape == (T, B, H)

    nc = _get_program()
    ident = np.eye(P, dtype=np.float32)
    in_maps = []
    for c in range(NCORES):
        fc = np.ascontiguousarray(f[:, c * BPC : (c + 1) * BPC, :]).reshape(T, N)
        zc = np.ascontiguousarray(z[:, c * BPC : (c + 1) * BPC, :]).reshape(T, N)
        in_maps.append({"f": fc, "z": zc, "ident": ident})

    kres = run_bass_kernel_spmd(
        nc, in_maps, list(range(NCORES)), trace=_trace
    )
    out = np.empty((T, B, H), dtype=np.float32)
    for c in range(NCORES):
        out[:, c * BPC : (c + 1) * BPC, :] = kres.results[c]["h"].reshape(T, BPC, H)
    if _trace:
        return out, kres
    return out


# revision 4
# speedup vs baseline: 1.0944x; 1.0944x over previous
"""ForgetMult (h_t = f_t*h_{t-1} + (1-f_t)*z_t) on 8 TRN2 NeuronCores.

Full inputs f, z: [T=1024, B=32, H=1024] f32. Output h: [T, B, H].

Sharding: batch dim across the 8 cores (4 batches/core), no communication.
Per core the problem is [T=1024, N=4096] with an independent linear
recurrence along T for each of the N columns.

Per-core dataflow (per n-group of W=512 columns):
  - one DMA per tensor brings the whole [T, W] panel in as a
    [128, T/128 * W] t-block-interleaved SBUF tile (2 KiB rows)
  - DVE: bneg = (f - 1) * z  (one scalar_tensor_tensor op; = -(1-f)*z)
  - PE: transpose 128x128 blocks of f (fp32) and bneg (bf16) into PSUM
  - ACT: copy f_tr PSUM->SBUF (scan operands can't both live in PSUM)
  - DVE: tensor_tensor_scan along t: state = f*state - bneg  (fp32 state)
  - PE: transpose h (bf16) back to [t-part, n-free] PSUM staging
  - ACT: copy h PSUM->SBUF panel (bf16 -> fp32 cast), one DMA out

bf16 is used only on the additive input (bneg) and the stored output; the
recurrence coefficients f and the scan state stay fp32, so quantization
does not compound across time steps.
"""

from contextlib import ExitStack

import numpy as np

T, B, H = 1024, 32, 1024
NCORES = 8
BPC = B // NCORES  # 4 batches per core
N = BPC * H  # 4096 recurrence columns per core
P = 128

W_FULL = 512  # panel width (columns per n-group)
USE_BF16 = True  # bf16 for bneg + h transpose paths (f/state stay fp32)


def build_forget_mult(tc, h_d, f_d, z_d, i_d, ib_d, ctx, t_sz, n_sz, w_sz,
                      use_bf16=USE_BF16):
    """Emit the per-core Tile program. f_d/z_d/h_d are DRAM APs [t_sz, n_sz]."""
    import concourse.bass as bass
    from concourse import mybir

    nc = tc.nc
    fp32 = mybir.dt.float32
    bf16 = mybir.dt.bfloat16
    cdt = bf16 if use_bf16 else fp32
    su = mybir.AluOpType.subtract
    mu = mybir.AluOpType.mult

    tb = t_sz // P  # t-blocks (8)
    ng = n_sz // w_sz  # n-groups (8)
    nb = w_sz // P  # n-blocks per group (4)
    n_halves = 2  # scan chunks along T
    thb = tb // n_halves  # t-blocks per half (4)
    scan_len = thb * P  # 512
    assert t_sz % P == 0 and n_sz % w_sz == 0 and w_sz % P == 0 and tb % 2 == 0

    const_pool = ctx.enter_context(tc.tile_pool(name="const", bufs=1))
    ident = const_pool.tile([P, P], fp32)
    nc.sync.dma_start(ident[:], i_d[:])
    identb = const_pool.tile([P, P], bf16)
    nc.sync.dma_start(identb[:], ib_d[:])
    id_of = {fp32: ident, bf16: identb}

    f_pool = ctx.enter_context(tc.tile_pool(name="fpanel", bufs=2))
    z_pool = ctx.enter_context(tc.tile_pool(name="zpanel", bufs=2))
    b_pool = ctx.enter_context(tc.tile_pool(name="bpanel", bufs=2))
    h_pool = ctx.enter_context(tc.tile_pool(name="hpanel", bufs=2))
    ftr_s_pool = ctx.enter_context(tc.tile_pool(name="ftrs", bufs=2))
    htr_pool = ctx.enter_context(tc.tile_pool(name="htr", bufs=2 * nb))
    ftr_p_pool = ctx.enter_context(tc.tile_pool(name="ftrp", bufs=2, space="PSUM"))
    btr_p_pool = ctx.enter_context(tc.tile_pool(name="btrp", bufs=2, space="PSUM"))
    hbk_p_pool = ctx.enter_context(tc.tile_pool(name="hbkp", bufs=thb, space="PSUM"))

    def panel_dram(d, g):
        # [t_sz, W] column slice viewed as [p, j, c] (j = t-block)
        return d[:, w_sz * g : w_sz * (g + 1)].rearrange(
            "(j p) c -> p j c", p=P
        )

    for g in range(ng):
        fp = f_pool.tile([P, tb, w_sz], fp32, tag="fpanel")
        nc.sync.dma_start(fp[:], panel_dram(f_d, g))
        zp = z_pool.tile([P, tb, w_sz], fp32, tag="zpanel")
        nc.sync.dma_start(zp[:], panel_dram(z_d, g))
        bp = b_pool.tile([P, tb, w_sz], cdt, tag="bpanel")
        hp = h_pool.tile([P, tb, w_sz], fp32, tag="hpanel")

        for j in range(tb):
            # bneg = (f - 1) * z
            nc.vector.scalar_tensor_tensor(
                bp[:, j], fp[:, j], 1.0, zp[:, j], op0=su, op1=mu
            )

        prev_htr = [None] * nb
        for half in range(n_halves):
            hbks = []
            for jj in range(thb):
                hbk = hbk_p_pool.tile([P, w_sz], cdt, tag="hbkp", name=f"hbk_{g}_{half}_{jj}")
                hbks.append(hbk)
            for i in range(nb):
                ftr_p = ftr_p_pool.tile([P, scan_len], fp32, tag="ftrp")
                btr_p = btr_p_pool.tile([P, scan_len], cdt, tag="btrp")
                for jj in range(thb):
                    j = half * thb + jj
                    nc.tensor.transpose(
                        ftr_p[:, P * jj : P * (jj + 1)],
                        fp[:, j, P * i : P * (i + 1)],
                        ident[:],
                    )
                    nc.tensor.transpose(
                        btr_p[:, P * jj : P * (jj + 1)],
                        bp[:, j, P * i : P * (i + 1)],
                        id_of[cdt][:],
                    )
                ftr_s = ftr_s_pool.tile([P, scan_len], fp32, tag="ftrs")
                nc.scalar.copy(ftr_s[:], ftr_p[:])
                htr = htr_pool.tile([P, scan_len], cdt, tag="htr")
                init = 0.0 if half == 0 else prev_htr[i][:, scan_len - 1 : scan_len]
                # state = (f * state) - bneg == f*state + (1-f)*z
                nc.vector.tensor_tensor_scan(
                    htr[:], ftr_s[:], btr_p[:], init, op0=mu, op1=su
                )
                prev_htr[i] = htr
                for jj in range(thb):
                    nc.tensor.transpose(
                        hbks[jj][:, P * i : P * (i + 1)],
                        htr[:, P * jj : P * (jj + 1)],
                        id_of[cdt][:],
                    )
            for jj in range(thb):
                j = half * thb + jj
                nc.scalar.copy(hp[:, j], hbks[jj][:])

        nc.sync.dma_start(panel_dram(h_d, g), hp[:])


def build_program(t_sz=T, n_sz=N, w_sz=W_FULL, use_bf16=USE_BF16):
    import concourse.tile as tile
    from concourse import bacc, mybir

    nc = bacc.Bacc(
        "TRN2",
        target_bir_lowering=False,
        debug=False,
        enable_asserts=False,
        num_devices=NCORES,
    )
    fp32 = mybir.dt.float32
    bf16 = mybir.dt.bfloat16
    f_d = nc.dram_tensor("f", [t_sz, n_sz], fp32, kind="ExternalInput").ap()
    z_d = nc.dram_tensor("z", [t_sz, n_sz], fp32, kind="ExternalInput").ap()
    i_d = nc.dram_tensor("ident", [P, P], fp32, kind="ExternalInput").ap()
    ib_d = nc.dram_tensor("identb", [P, P], bf16, kind="ExternalInput").ap()
    h_d = nc.dram_tensor("h", [t_sz, n_sz], fp32, kind="ExternalOutput").ap()
    with tile.TileContext(nc) as tc:
        with ExitStack() as ctx:
            build_forget_mult(
                tc, h_d, f_d, z_d, i_d, ib_d, ctx, t_sz, n_sz, w_sz, use_bf16
            )
    nc.compile()
    return nc


_compiled = None


def _get_program():
    global _compiled
    if _compiled is None:
        _compiled = build_program()
    return _compiled


def kernel(f, z, _trace=False):
    import ml_dtypes
    from concourse.bass_utils import run_bass_kernel_spmd

    f = np.asarray(f, dtype=np.float32)
    z = np.asarray(z, dtype=np.float32)
    assert f.shape == (T, B, H) and z.shape == (T, B, H)

    nc = _get_program()
    ident = np.eye(P, dtype=np.float32)
    identb = np.eye(P).astype(ml_dtypes.bfloat16)
    in_maps = []
    for c in range(NCORES):
        fc = np.ascontiguousarray(f[:, c * BPC : (c + 1) * BPC, :]).reshape(T, N)
        zc = np.ascontiguousarray(z[:, c * BPC : (c + 1) * BPC, :]).reshape(T, N)
        in_maps.append({"f": fc, "z": zc, "ident": ident, "identb": identb})

    kres = run_bass_kernel_spmd(nc, in_maps, list(range(NCORES)), trace=_trace)
    out = np.empty((T, B, H), dtype=np.float32)
    for c in range(NCORES):
        out[:, c * BPC : (c + 1) * BPC, :] = kres.results[c]["h"].reshape(T, BPC, H)
    if _trace:
        return out, kres
    return out
